# revision 2
# baseline (speedup 1.0000x reference)
"""Trainium2 Bass kernel for nn_CrossAttention (FFT-query cross attention).

Math:
  out = softmax((Re(FFT(query, axis=1)) @ Wq^T + bq) @ (key @ Wk^T + bk)^T / sqrt(D)) @ key

Key identities used:
  * Re(FFT(x))[j] = sum_n x[n] cos(2*pi*j*n/N): a matmul with a cosine matrix.
  * cos rows satisfy C[N-j] = C[j]  =>  q rows mirror:  q[j] == q[N-j].
    The whole downstream pipeline is row-wise in q, so out[b, j] == out[b, N-j].
    Only rows j = 0..1024 are computed on device (padded to 1152 = 9*128);
    rows 1025..2047 are mirrored from rows 1023..1 on the host.
  * cos cols satisfy C[:, n] = C[:, N-n]  =>  fold x into
    y[0] = x[0], y[n] = x[n] + x[N-n] (n=1..1023), y[1024] = x[1024]
    and contract over only 1025 terms (plus one ones-row for the bq bias).
  * bk drops out of softmax entirely (adds a per-query-row constant to scores).
  * The 1/sqrt(D) scale is folded into the cosine table.
  * 1/rowsum of softmax is applied to the final [128, 256] output tiles, not
    to the [128, 2048] probability tiles.

Per-core layout (core b handles batch b; 8 cores, 8 batches):
  MM-A: z[n, d]   = y @ Wq^T            lhsT = y^T (host),   rhs = Wq^T (host)
  MM-B: qsT[d, j] = z^T @ (C/16)        lhsT = z,            rhs = cos table (host)
  MM-C: kT[d, nk] = Wk @ key^T          lhsT = Wk^T (host),  rhs = key^T (host)
  MM-D: S[j, nk]  = qs @ k^T            lhsT = qsT,          rhs = kT
  softmax rows of S (on-chip, two 1024-wide halves; exp via ACT with accum_out)
  MM-T: P^T tiles via PE transpose
  MM-E: o[j, d]   = P @ key             lhsT = P^T tiles,    rhs = key (host)

All matmul operands are float32r (TF32-like PE mode, ~1.6e-4 matmul rel err,
1 cycle/row at free-dim >= 256 vs 4 for plain fp32).
"""

import numpy as np

import concourse.bass as bass
import concourse.tile as tile
from concourse import bacc, mybir
from concourse.bass_utils import run_bass_kernel_spmd

B = 8
NSEQ = 2048          # query/key sequence length
D = 256              # feature dim
NQH = 1152           # computed query rows (9 tiles of 128; rows >1024 unused)
NFOLD = 1026         # folded contraction: 1025 cosine rows + 1 bias row
NJT = NQH // 128     # 9 query-row tiles
NKT = NSEQ // 128    # 16 key tiles
SCALE = 1.0 / 16.0   # 1/sqrt(D)

f32 = mybir.dt.float32
f32r = mybir.dt.float32r

_compiled = {}


def _build_module(trace_label="k"):
    nc = bacc.Bacc("TRN2", target_bir_lowering=False, debug=False, num_devices=B)

    dram = {}
    def din(name, shape):
        dram[name] = nc.dram_tensor(name, list(shape), f32, kind="ExternalInput").ap()
    def dout(name, shape):
        dram[name] = nc.dram_tensor(name, list(shape), f32, kind="ExternalOutput").ap()

    din("yt", (D, 1025))        # folded query, transposed
    din("bq", (1, D))
    din("wqt", (D, D))          # Wq^T
    din("wkt", (D, D))          # Wk^T
    din("keyt", (D, NSEQ))      # key^T
    din("keyn", (NSEQ, D))      # key natural
    din("ct", (NFOLD, NQH))     # cosine table * SCALE (+ ones*SCALE bias row)
    din("ident", (128, 128))
    dout("ob", (NQH, D))

    with tile.TileContext(nc) as tc:
        _emit(nc, tc, dram)
    nc.compile()
    return nc


def _emit(nc, tc, dram):
    from contextlib import ExitStack

    with ExitStack() as ctx:
        const = ctx.enter_context(tc.tile_pool(name="const", bufs=1))
        zpool = ctx.enter_context(tc.tile_pool(name="z", bufs=1))
        qkpool = ctx.enter_context(tc.tile_pool(name="qk", bufs=1))

        # ---- constant loads ----
        yt = [const.tile([128, 1025], f32r, tag=f"yt{i}", name=f"yt{i}") for i in range(2)]
        for i in range(2):
            nc.sync.dma_start(yt[i][:], dram["yt"][i * 128:(i + 1) * 128, :].bitcast(f32r))
        wqt = [const.tile([128, D], f32r, tag=f"wqt{i}", name=f"wqt{i}") for i in range(2)]
        wkt = [const.tile([128, D], f32r, tag=f"wkt{i}", name=f"wkt{i}") for i in range(2)]
        for i in range(2):
            nc.sync.dma_start(wqt[i][:], dram["wqt"][i * 128:(i + 1) * 128, :].bitcast(f32r))
            nc.sync.dma_start(wkt[i][:], dram["wkt"][i * 128:(i + 1) * 128, :].bitcast(f32r))
        keyt = [const.tile([128, NSEQ], f32r, tag=f"keyt{i}", name=f"keyt{i}") for i in range(2)]
        for i in range(2):
            nc.sync.dma_start(keyt[i][:], dram["keyt"][i * 128:(i + 1) * 128, :].bitcast(f32r))
        keyn = [const.tile([128, D], f32r, tag=f"keyn{i}", name=f"keyn{i}") for i in range(NKT)]
        for i in range(NKT):
            nc.sync.dma_start(keyn[i][:], dram["keyn"][i * 128:(i + 1) * 128, :].bitcast(f32r))
        cts = []
        for i in range(9):
            r = 128 if i < 8 else 2
            t = const.tile([r, NQH], f32r, tag=f"ct{i}", name=f"ct{i}")
            nc.sync.dma_start(t[:], dram["ct"][i * 128:i * 128 + r, :].bitcast(f32r))
            cts.append(t)
        id_r = const.tile([128, 128], f32r, tag="ident", name="ident")
        nc.sync.dma_start(id_r[:], dram["ident"][:].bitcast(f32r))

        # ---- phase A: z = y @ Wq^T  (9 row tiles: 8 full + 1 single row) ----
        zbuf = []
        for i in range(8):
            zbuf.append(zpool.tile([128, D], f32r, tag=f"z{i}", name=f"z{i}"))
        zbuf.append(zpool.tile([2, D], f32r, tag="z8", name="z8"))  # row0: z[1024], row1: bq
        nc.sync.dma_start(zbuf[8][1:2, :], dram["bq"][:].bitcast(f32r))

        with tc.tile_pool(name="psA", bufs=2, space="PSUM") as psA:
            for nt in range(9):
                m = 128 if nt < 8 else 1
                ps = psA.tile([128, D], f32, tag="psA", name="psA")
                for kd in range(2):
                    nc.tensor.matmul(
                        ps[:m, :], yt[kd][:, nt * 128:nt * 128 + m], wqt[kd][:],
                        start=(kd == 0), stop=(kd == 1))
                nc.vector.tensor_copy(zbuf[nt][:m, :], ps[:m, :])

        # ---- phase B: qsT = z^T @ (C/16)  [256, 1152] ----
        qsT = [qkpool.tile([128, NQH], f32r, tag=f"qsT{i}", name=f"qsT{i}") for i in range(2)]
        with tc.tile_pool(name="psB", bufs=2, space="PSUM") as psB:
            for dt in range(2):
                for c in range(3):  # 384-wide chunks
                    sl = slice(c * 384, (c + 1) * 384)
                    ps = psB.tile([128, 384], f32, tag="psB", name="psB")
                    for kt in range(9):
                        kr = 128 if kt < 8 else 2
                        nc.tensor.matmul(
                            ps[:], zbuf[kt][:kr, dt * 128:(dt + 1) * 128],
                            cts[kt][:kr, sl], start=(kt == 0), stop=(kt == 8))
                    nc.vector.tensor_copy(qsT[dt][:, sl], ps[:])

        # ---- phase C: kT = Wk @ key^T  [256, 2048] ----
        kT = [qkpool.tile([128, NSEQ], f32r, tag=f"kT{i}", name=f"kT{i}") for i in range(2)]
        with tc.tile_pool(name="psC", bufs=2, space="PSUM") as psC:
            for dt in range(2):
                for c in range(4):  # 512-wide chunks
                    sl = slice(c * 512, (c + 1) * 512)
                    ps = psC.tile([128, 512], f32, tag="psC", name="psC")
                    for kd in range(2):
                        nc.tensor.matmul(
                            ps[:], wkt[kd][:, dt * 128:(dt + 1) * 128],
                            keyt[kd][:, sl], start=(kd == 0), stop=(kd == 1))
                    nc.vector.tensor_copy(kT[dt][:, sl], ps[:])

        # ---- phase D: attention over 9 query tiles, software-pipelined ----
        with ExitStack() as jctx:
            psS = jctx.enter_context(tc.tile_pool(name="psS", bufs=2, space="PSUM"))
            psT = jctx.enter_context(tc.tile_pool(name="psT", bufs=2, space="PSUM"))
            psO = jctx.enter_context(tc.tile_pool(name="psO", bufs=2, space="PSUM"))
            work = jctx.enter_context(tc.tile_pool(name="work", bufs=2))
            ptp = jctx.enter_context(tc.tile_pool(name="ptp", bufs=4))
            stats = jctx.enter_context(tc.tile_pool(name="stats", bufs=4))

            state = {}  # per-jt carried tiles
            for step in range(NJT + 1):
                if step < NJT:
                    jt = step
                    jsl = slice(jt * 128, (jt + 1) * 128)
                    # scores in two 1024-wide halves (2 psum banks each)
                    halves = []
                    for h in range(2):
                        sh = psS.tile([128, 1024], f32, tag="psS", name="psS")
                        for c in range(2):
                            sl = slice(c * 512, (c + 1) * 512)
                            ksl = slice(h * 1024 + c * 512, h * 1024 + (c + 1) * 512)
                            for dt in range(2):
                                nc.tensor.matmul(
                                    sh[:, sl], qsT[dt][:, jsl], kT[dt][:, ksl],
                                    start=(dt == 0), stop=(dt == 1))
                        halves.append(sh)
                    mx = [stats.tile([128, 1], f32, tag=f"mx{h}", name=f"mx{h}") for h in range(2)]
                    for h in range(2):
                        nc.vector.reduce_max(out=mx[h][:], in_=halves[h][:],
                                             axis=mybir.AxisListType.X)
                    rmax = stats.tile([128, 1], f32, tag="rmax", name="rmax")
                    nc.vector.tensor_scalar_max(rmax[:], mx[0][:], mx[1][:])
                    negmax = stats.tile([128, 1], f32, tag="negmax", name="negmax")
                    nc.scalar.mul(negmax[:], rmax[:], -1.0)
                    p_t = work.tile([128, NSEQ], f32r, tag="p", name="p")
                    sm = [stats.tile([128, 1], f32, tag=f"sm{h}", name=f"sm{h}") for h in range(2)]
                    for h in range(2):
                        nc.scalar.activation(
                            out=p_t[:, h * 1024:(h + 1) * 1024], in_=halves[h][:],
                            func=mybir.ActivationFunctionType.Exp,
                            bias=negmax[:], scale=1.0, accum_out=sm[h][:])
                    rsum = stats.tile([128, 1], f32, tag="rsum", name="rsum")
                    nc.vector.tensor_scalar_add(rsum[:], sm[0][:], sm[1][:])
                    recip = stats.tile([128, 1], f32, tag="recip", name="recip")
                    nc.vector.reciprocal(recip[:], rsum[:])
                    state[jt] = (p_t, recip)

                if step >= 1:
                    jt = step - 1
                    p_t, recip = state.pop(jt)
                    po = psO.tile([128, D], f32, tag="psO", name="psO")
                    for kt in range(NKT):
                        pt_ps = psT.tile([128, 128], f32r, tag="psT", name="psT")
                        nc.tensor.matmul(pt_ps[:], p_t[:, kt * 128:(kt + 1) * 128],
                                         id_r[:], is_transpose=True,
                                         start=True, stop=True)
                        pt_sb = ptp.tile([128, 128], f32r, tag="pt", name="pt")
                        nc.vector.tensor_copy(pt_sb[:], pt_ps[:])
                        nc.tensor.matmul(po[:], pt_sb[:], keyn[kt][:],
                                         start=(kt == 0), stop=(kt == NKT - 1))
                    osb = work.tile([128, D], f32, tag="osb", name="osb")
                    nc.vector.tensor_scalar_mul(osb[:], po[:], recip[:])
                    nc.sync.dma_start(dram["ob"][jt * 128:(jt + 1) * 128, :], osb[:])


def _host_prep(query, key, Wq, bq, Wk, bk):
    """Build per-core input maps (fold+transpose query, transpose key/weights,
    cosine table)."""
    query = np.ascontiguousarray(query, dtype=np.float32)
    key = np.ascontiguousarray(key, dtype=np.float32)

    nn = np.arange(NFOLD - 1, dtype=np.float64)          # 0..1024
    jj = np.arange(NQH, dtype=np.float64)
    ct = np.empty((NFOLD, NQH), dtype=np.float32)
    ct[:-1] = (np.cos(2.0 * np.pi * np.outer(nn, jj) / NSEQ) * SCALE).astype(np.float32)
    ct[-1] = SCALE  # bias row (ones * scale)

    wqt = np.ascontiguousarray(Wq.T, dtype=np.float32)
    wkt = np.ascontiguousarray(Wk.T, dtype=np.float32)
    bq2 = np.ascontiguousarray(bq.reshape(1, D), dtype=np.float32)
    ident = np.eye(128, dtype=np.float32)

    in_maps = []
    for b in range(B):
        x = query[b]
        y = np.empty((1025, D), dtype=np.float32)
        y[0] = x[0]
        y[1:1024] = x[1:1024] + x[2047:1024:-1]
        y[1024] = x[1024]
        in_maps.append({
            "yt": np.ascontiguousarray(y.T),
            "bq": bq2,
            "wqt": wqt,
            "wkt": wkt,
            "keyt": np.ascontiguousarray(key[b].T),
            "keyn": np.ascontiguousarray(key[b]),
            "ct": ct,
            "ident": ident,
        })
    return in_maps


def kernel(query, key, Wq, bq, Wk, bk, _trace=False, _trace_kwargs=None):
    if "nc" not in _compiled:
        _compiled["nc"] = _build_module()
    nc = _compiled["nc"]

    in_maps = _host_prep(query, key, Wq, bq, Wk, bk)
    kw = {}
    if _trace:
        kw["trace"] = True
        if _trace_kwargs:
            kw.update(_trace_kwargs)
    res = run_bass_kernel_spmd(nc, in_maps, core_ids=list(range(B)), **kw)
    _compiled["last_results"] = res

    out = np.empty((B, NSEQ, D), dtype=np.float32)
    for b in range(B):
        ob = res.results[b]["ob"]
        out[b, :1025] = ob[:1025]
        out[b, 1025:] = ob[1023:0:-1]
    return out


# revision 4
# speedup vs baseline: 1.0780x; 1.0780x over previous
"""Trainium2 Bass kernel for nn_CrossAttention (FFT-query cross attention).

Math:
  out = softmax((Re(FFT(query, axis=1)) @ Wq^T + bq) @ (key @ Wk^T + bk)^T / sqrt(D)) @ key

Key identities used:
  * Re(FFT(x))[j] = sum_n x[n] cos(2*pi*j*n/N): a matmul with a cosine matrix.
  * cos rows satisfy C[N-j] = C[j]  =>  q rows mirror:  q[j] == q[N-j].
    The whole downstream pipeline is row-wise in q, so out[b, j] == out[b, N-j].
    Only rows j = 0..1024 are computed on device (padded to 1152 = 9*128);
    rows 1025..2047 are mirrored from rows 1023..1 on the host.
  * cos cols satisfy C[:, n] = C[:, N-n]  =>  fold x into
    y[0] = x[0], y[n] = x[n] + x[N-n] (n=1..1023), y[1024] = x[1024]
    and contract over only 1025 terms (plus one ones-row for the bq bias).
  * bk drops out of softmax entirely (adds a per-query-row constant to scores).
  * The 1/sqrt(D) scale is folded into the cosine table.
  * 1/rowsum of softmax is applied to the final [128, 256] output tiles, not
    to the [128, 2048] probability tiles.

Per-core layout (core b handles batch b; 8 cores, 8 batches):
  MM-A: z[n, d]   = y @ Wq^T            lhsT = y^T (host),   rhs = Wq^T (host)
  MM-B: qsT[d, j] = z^T @ (C/16)        lhsT = z,            rhs = cos table (host)
  MM-C: kT[d, nk] = Wk @ key^T          lhsT = Wk^T (host),  rhs = key^T (host)
  MM-D: S[j, nk]  = qs @ k^T            lhsT = qsT,          rhs = kT
  softmax rows of S (on-chip, two 1024-wide halves; exp via ACT with accum_out,
  P written in bf16)
  P^T tiles via DMA transpose (bf16, SBUF->SBUF; no PE transposes, no casts)
  MM-E: o[j, d]   = P @ key             lhsT = P^T (bf16),   rhs = key bf16 (host)

Scores-path matmuls in float32r (TF32-like PE mode, ~1.6e-4 matmul rel err,
1 cycle/row at free-dim >= 256 vs 4 for plain fp32); probability/value
matmul in bf16 (P in [0,1], value contraction tolerates 2^-8).
"""

import numpy as np
import ml_dtypes

import concourse.bass as bass
import concourse.tile as tile
from concourse import bacc, mybir
from concourse.bass_utils import run_bass_kernel_spmd

B = 8
NSEQ = 2048          # query/key sequence length
D = 256              # feature dim
NQH = 1152           # computed query rows (9 tiles of 128; rows >1024 unused)
NFOLD = 1026         # folded contraction: 1025 cosine rows + 1 bias row
NJT = NQH // 128     # 9 query-row tiles
NKT = NSEQ // 128    # 16 key tiles
SCALE = 1.0 / 16.0   # 1/sqrt(D)

f32 = mybir.dt.float32
f32r = mybir.dt.float32r
bf16 = mybir.dt.bfloat16
fp16 = mybir.dt.float16

_compiled = {}


def _build_module():
    nc = bacc.Bacc("TRN2", target_bir_lowering=False, debug=False, num_devices=B)

    dram = {}
    def din(name, shape, dt=f32):
        dram[name] = nc.dram_tensor(name, list(shape), dt, kind="ExternalInput").ap()
    def dout(name, shape):
        dram[name] = nc.dram_tensor(name, list(shape), f32, kind="ExternalOutput").ap()

    din("yt", (D, 1025))        # folded query, transposed
    din("bq", (1, D))
    din("wqt", (D, D))          # Wq^T
    din("wkt", (D, D))          # Wk^T
    din("keyt", (D, NSEQ))      # key^T
    din("keyn", (NSEQ, D), bf16)  # key natural, bf16 (value side)
    din("ct", (NFOLD, NQH))     # cosine table * SCALE (+ ones*SCALE bias row)
    din("ident", (128, 128), bf16)
    dout("ob", (NQH, D))

    with tile.TileContext(nc) as tc:
        _emit(nc, tc, dram)
    nc.compile()
    return nc


def _emit(nc, tc, dram):
    from contextlib import ExitStack

    with ExitStack() as ctx:
        const = ctx.enter_context(tc.tile_pool(name="const", bufs=1))
        zpool = ctx.enter_context(tc.tile_pool(name="z", bufs=1))
        qkpool = ctx.enter_context(tc.tile_pool(name="qk", bufs=1))

        # ---- constant loads, in phase-consumption order ----
        yt = [const.tile([128, 1025], f32r, tag=f"yt{i}", name=f"yt{i}") for i in range(2)]
        wqt = [const.tile([128, D], f32r, tag=f"wqt{i}", name=f"wqt{i}") for i in range(2)]
        for i in range(2):
            nc.sync.dma_start(yt[i][:], dram["yt"][i * 128:(i + 1) * 128, :].bitcast(f32r))
            nc.sync.dma_start(wqt[i][:], dram["wqt"][i * 128:(i + 1) * 128, :].bitcast(f32r))
        cts = []
        for i in range(9):
            r = 128 if i < 8 else 2
            t = const.tile([r, NQH], f32r, tag=f"ct{i}", name=f"ct{i}")
            nc.sync.dma_start(t[:], dram["ct"][i * 128:i * 128 + r, :].bitcast(f32r))
            cts.append(t)
        wkt = [const.tile([128, D], f32r, tag=f"wkt{i}", name=f"wkt{i}") for i in range(2)]
        keyt = [const.tile([128, NSEQ], f32r, tag=f"keyt{i}", name=f"keyt{i}") for i in range(2)]
        for i in range(2):
            nc.sync.dma_start(wkt[i][:], dram["wkt"][i * 128:(i + 1) * 128, :].bitcast(f32r))
            nc.sync.dma_start(keyt[i][:], dram["keyt"][i * 128:(i + 1) * 128, :].bitcast(f32r))
        keyn = [const.tile([128, D], bf16, tag=f"keyn{i}", name=f"keyn{i}") for i in range(NKT)]
        for i in range(NKT):
            nc.sync.dma_start(keyn[i][:], dram["keyn"][i * 128:(i + 1) * 128, :])
        id_b = const.tile([128, 128], bf16, tag="ident", name="ident")
        nc.sync.dma_start(id_b[:], dram["ident"][:])

        # ---- phase A: z = y @ Wq^T  (9 row tiles: 8 full + 1 single row) ----
        zbuf = []
        for i in range(8):
            zbuf.append(zpool.tile([128, D], f32r, tag=f"z{i}", name=f"z{i}"))
        zbuf.append(zpool.tile([2, D], f32r, tag="z8", name="z8"))  # row0: z[1024], row1: bq
        nc.sync.dma_start(zbuf[8][1:2, :], dram["bq"][:].bitcast(f32r))

        with tc.tile_pool(name="psA", bufs=2, space="PSUM") as psA:
            for nt in range(9):
                m = 128 if nt < 8 else 1
                ps = psA.tile([128, D], f32, tag="psA", name="psA")
                for kd in range(2):
                    nc.tensor.matmul(
                        ps[:m, :], yt[kd][:, nt * 128:nt * 128 + m], wqt[kd][:],
                        start=(kd == 0), stop=(kd == 1))
                nc.vector.tensor_copy(zbuf[nt][:m, :], ps[:m, :])

        # ---- phase B: qsT = z^T @ (C/16)  [256, 1152] ----
        qsT = [qkpool.tile([128, NQH], fp16, tag=f"qsT{i}", name=f"qsT{i}") for i in range(2)]
        with tc.tile_pool(name="psB", bufs=2, space="PSUM") as psB:
            for dt in range(2):
                for c in range(3):  # 384-wide chunks
                    sl = slice(c * 384, (c + 1) * 384)
                    ps = psB.tile([128, 384], f32, tag="psB", name="psB")
                    for kt in range(9):
                        kr = 128 if kt < 8 else 2
                        nc.tensor.matmul(
                            ps[:], zbuf[kt][:kr, dt * 128:(dt + 1) * 128],
                            cts[kt][:kr, sl], start=(kt == 0), stop=(kt == 8))
                    nc.vector.tensor_copy(qsT[dt][:, sl], ps[:])

        # ---- phase C: kT = Wk @ key^T  [256, 2048] ----
        kT = [qkpool.tile([128, NSEQ], fp16, tag=f"kT{i}", name=f"kT{i}") for i in range(2)]
        with tc.tile_pool(name="psC", bufs=2, space="PSUM") as psC:
            for dt in range(2):
                for c in range(4):  # 512-wide chunks
                    sl = slice(c * 512, (c + 1) * 512)
                    ps = psC.tile([128, 512], f32, tag="psC", name="psC")
                    for kd in range(2):
                        nc.tensor.matmul(
                            ps[:], wkt[kd][:, dt * 128:(dt + 1) * 128],
                            keyt[kd][:, sl], start=(kd == 0), stop=(kd == 1))
                    nc.vector.tensor_copy(kT[dt][:, sl], ps[:])

        # ---- phase D: attention over 9 query tiles, software-pipelined ----
        with ExitStack() as jctx:
            psS = jctx.enter_context(tc.tile_pool(name="psS", bufs=2, space="PSUM"))
            psT = jctx.enter_context(tc.tile_pool(name="psT", bufs=2, space="PSUM"))
            psO = jctx.enter_context(tc.tile_pool(name="psO", bufs=2, space="PSUM"))
            work = jctx.enter_context(tc.tile_pool(name="work", bufs=2))
            ptp = jctx.enter_context(tc.tile_pool(name="ptp", bufs=8))
            stats = jctx.enter_context(tc.tile_pool(name="stats", bufs=4))

            state = {}  # per-jt carried tiles
            for step in range(NJT + 1):
                if step < NJT:
                    jt = step
                    jsl = slice(jt * 128, (jt + 1) * 128)
                    # scores in two 1024-wide halves (2 psum banks each)
                    halves = []
                    for h in range(2):
                        sh = psS.tile([128, 1024], f32, tag="psS", name="psS")
                        for c in range(2):
                            sl = slice(c * 512, (c + 1) * 512)
                            ksl = slice(h * 1024 + c * 512, h * 1024 + (c + 1) * 512)
                            for dt in range(2):
                                nc.tensor.matmul(
                                    sh[:, sl], qsT[dt][:, jsl], kT[dt][:, ksl],
                                    start=(dt == 0), stop=(dt == 1))
                        halves.append(sh)
                    mx = [stats.tile([128, 1], f32, tag=f"mx{h}", name=f"mx{h}") for h in range(2)]
                    for h in range(2):
                        nc.vector.reduce_max(out=mx[h][:], in_=halves[h][:],
                                             axis=mybir.AxisListType.X)
                    rmax = stats.tile([128, 1], f32, tag="rmax", name="rmax")
                    nc.vector.tensor_scalar_max(rmax[:], mx[0][:], mx[1][:])
                    negmax = stats.tile([128, 1], f32, tag="negmax", name="negmax")
                    nc.scalar.mul(negmax[:], rmax[:], -1.0)
                    p_t = work.tile([128, NSEQ], bf16, tag="p", name="p")
                    sm = [stats.tile([128, 1], f32, tag=f"sm{h}", name=f"sm{h}") for h in range(2)]
                    for h in range(2):
                        nc.scalar.activation(
                            out=p_t[:, h * 1024:(h + 1) * 1024], in_=halves[h][:],
                            func=mybir.ActivationFunctionType.Exp,
                            bias=negmax[:], scale=1.0, accum_out=sm[h][:])
                    rsum = stats.tile([128, 1], f32, tag="rsum", name="rsum")
                    nc.vector.tensor_scalar_add(rsum[:], sm[0][:], sm[1][:])
                    recip = stats.tile([128, 1], f32, tag="recip", name="recip")
                    nc.vector.reciprocal(recip[:], rsum[:])
                    state[jt] = (p_t, recip)

                if step >= 1:
                    jt = step - 1
                    p_t, recip = state.pop(jt)
                    po = psO.tile([128, D], f32, tag="psO", name="psO")
                    for kt in range(NKT):
                        pt_ps = psT.tile([128, 128], bf16, tag="psT", name="psT")
                        nc.tensor.matmul(pt_ps[:], p_t[:, kt * 128:(kt + 1) * 128],
                                         id_b[:], is_transpose=True,
                                         start=True, stop=True)
                        pt_sb = ptp.tile([128, 128], bf16, tag="pt", name="pt")
                        nc.vector.tensor_copy(pt_sb[:], pt_ps[:])
                        nc.tensor.matmul(po[:], pt_sb[:], keyn[kt][:],
                                         start=(kt == 0), stop=(kt == NKT - 1))
                    osb = work.tile([128, D], f32, tag="osb", name="osb")
                    nc.vector.tensor_scalar_mul(osb[:], po[:], recip[:])
                    nc.sync.dma_start(dram["ob"][jt * 128:(jt + 1) * 128, :], osb[:])


def _host_prep(query, key, Wq, bq, Wk, bk):
    """Build per-core input maps (fold+transpose query, transpose key/weights,
    cosine table)."""
    query = np.ascontiguousarray(query, dtype=np.float32)
    key = np.ascontiguousarray(key, dtype=np.float32)

    nn = np.arange(NFOLD - 1, dtype=np.float64)          # 0..1024
    jj = np.arange(NQH, dtype=np.float64)
    ct = np.empty((NFOLD, NQH), dtype=np.float32)
    ct[:-1] = (np.cos(2.0 * np.pi * np.outer(nn, jj) / NSEQ) * SCALE).astype(np.float32)
    ct[-1] = SCALE  # bias row (ones * scale)

    wqt = np.ascontiguousarray(Wq.T, dtype=np.float32)
    wkt = np.ascontiguousarray(Wk.T, dtype=np.float32)
    bq2 = np.ascontiguousarray(bq.reshape(1, D), dtype=np.float32)

    in_maps = []
    for b in range(B):
        x = query[b]
        y = np.empty((1025, D), dtype=np.float32)
        y[0] = x[0]
        y[1:1024] = x[1:1024] + x[2047:1024:-1]
        y[1024] = x[1024]
        in_maps.append({
            "yt": np.ascontiguousarray(y.T),
            "bq": bq2,
            "wqt": wqt,
            "wkt": wkt,
            "keyt": np.ascontiguousarray(key[b].T),
            "keyn": np.ascontiguousarray(key[b]).astype(ml_dtypes.bfloat16),
            "ct": ct,
            "ident": np.eye(128, dtype=ml_dtypes.bfloat16),
        })
    return in_maps


def kernel(query, key, Wq, bq, Wk, bk, _trace=False, _trace_kwargs=None):
    if "nc" not in _compiled:
        _compiled["nc"] = _build_module()
    nc = _compiled["nc"]

    in_maps = _host_prep(query, key, Wq, bq, Wk, bk)
    kw = {}
    if _trace:
        kw["trace"] = True
        if _trace_kwargs:
            kw.update(_trace_kwargs)
    res = run_bass_kernel_spmd(nc, in_maps, core_ids=list(range(B)), **kw)
    _compiled["last_results"] = res

    out = np.empty((B, NSEQ, D), dtype=np.float32)
    for b in range(B):
        ob = res.results[b]["ob"]
        out[b, :1025] = ob[:1025]
        out[b, 1025:] = ob[1023:0:-1]
    return out


# revision 6
# speedup vs baseline: 1.1779x; 1.0926x over previous
"""Trainium2 Bass kernel for nn_CrossAttention (FFT-query cross attention).

Math:
  out = softmax((Re(FFT(query, axis=1)) @ Wq^T + bq) @ (key @ Wk^T + bk)^T / sqrt(D)) @ key

Key identities used:
  * Re(FFT(x))[j] = sum_n x[n] cos(2*pi*j*n/N): a matmul with a cosine matrix.
  * cos rows satisfy C[N-j] = C[j]  =>  q rows mirror:  q[j] == q[N-j].
    The whole downstream pipeline is row-wise in q, so out[b, j] == out[b, N-j].
    Only rows j = 0..1024 are computed on device (padded to 1152 = 9*128);
    rows 1025..2047 are mirrored from rows 1023..1 on the host.
  * cos cols satisfy C[:, n] = C[:, N-n]  =>  fold x into
    y[0] = x[0], y[n] = x[n] + x[N-n] (n=1..1023), y[1024] = x[1024]
    and contract over only 1025 terms (plus one ones-row for the bq bias).
  * bk drops out of softmax entirely (adds a per-query-row constant to scores).
  * The 1/sqrt(D) scale is folded into the cosine table.
  * 1/rowsum of softmax is applied to the final [128, 256] output tiles, not
    to the [128, 2048] probability tiles.

Per-core layout (core b handles batch b; 8 cores, 8 batches):
  MM-A: z[n, d]   = y @ Wq^T            lhsT = y^T (host),   rhs = Wq^T (host)
  MM-C: kT[d, nk] = Wk @ key^T          lhsT = Wk^T (host),  rhs = key^T (host)
  MM-B: qsT[d, j] = z^T @ (C/16)        lhsT = z,            rhs = cos table (host)
  MM-D: S[j, nk]  = qs @ k^T            lhsT = qsT,          rhs = kT
  softmax rows of S (two 1024-wide halves; exp via ACT with accum_out, P bf16)
  MM-T: P^T tiles via PE transpose (bf16)
  MM-E: o[j, d]   = P @ key             lhsT = P^T (bf16),   rhs = key bf16

Perf notes:
  * Everything scores-side is fp16 (11-bit mantissa, same precision class as
    the PE's f32r mode, but half the DMA bytes and FWL-capable weight loads).
  * P / value side is bf16: bf16 keeps fp32's exponent range, so tiny softmax
    tail probabilities don't flush to zero the way fp16 denormals would.
  * Matmul accumulation chains are interleaved across PSUM banks: consecutive
    PE instructions always target different banks so the drain of one overlaps
    the fill of the next (same-bank accumulation steps serialize).
"""

import numpy as np
import ml_dtypes

import concourse.bass as bass
import concourse.tile as tile
from concourse import bacc, mybir
from concourse.bass_utils import run_bass_kernel_spmd

B = 8
NSEQ = 2048          # query/key sequence length
D = 256              # feature dim
NQH = 1152           # computed query rows (9 tiles of 128; rows >1024 unused)
NFOLD = 1026         # folded contraction: 1025 cosine rows + 1 bias row
NJT = NQH // 128     # 9 query-row tiles
NKT = NSEQ // 128    # 16 key tiles
SCALE = 1.0 / 16.0   # 1/sqrt(D)

f32 = mybir.dt.float32
bf16 = mybir.dt.bfloat16
fp16 = mybir.dt.float16

_compiled = {}


def _build_module():
    nc = bacc.Bacc("TRN2", target_bir_lowering=False, debug=False, num_devices=B)

    dram = {}
    def din(name, shape, dt=fp16):
        dram[name] = nc.dram_tensor(name, list(shape), dt, kind="ExternalInput").ap()
    def dout(name, shape):
        dram[name] = nc.dram_tensor(name, list(shape), f32, kind="ExternalOutput").ap()

    din("yt", (D, 1025))          # folded query, transposed
    din("bq", (1, D))
    din("wqt", (D, D))            # Wq^T
    din("wkt", (D, D))            # Wk^T
    din("keyt", (D, NSEQ))        # key^T
    din("keyn", (NSEQ, D), bf16)  # key natural, bf16 (value side)
    din("ct", (NFOLD, NQH))       # cosine table * SCALE (+ ones*SCALE bias row)
    din("ident", (128, 128), bf16)
    dout("ob", (NQH, D))

    with tile.TileContext(nc) as tc:
        _emit(nc, tc, dram)
    nc.compile()
    return nc


def _emit(nc, tc, dram):
    from contextlib import ExitStack

    with ExitStack() as ctx:
        const = ctx.enter_context(tc.tile_pool(name="const", bufs=1))
        zpool = ctx.enter_context(tc.tile_pool(name="z", bufs=1))
        qkpool = ctx.enter_context(tc.tile_pool(name="qk", bufs=1))

        # ---- constant loads, in phase-consumption order (A, C, B, loop) ----
        yt = [const.tile([128, 1025], fp16, tag=f"yt{i}", name=f"yt{i}") for i in range(2)]
        wqt = [const.tile([128, D], fp16, tag=f"wqt{i}", name=f"wqt{i}") for i in range(2)]
        for i in range(2):
            nc.sync.dma_start(yt[i][:], dram["yt"][i * 128:(i + 1) * 128, :])
            nc.sync.dma_start(wqt[i][:], dram["wqt"][i * 128:(i + 1) * 128, :])
        wkt = [const.tile([128, D], fp16, tag=f"wkt{i}", name=f"wkt{i}") for i in range(2)]
        keyt = [const.tile([128, NSEQ], fp16, tag=f"keyt{i}", name=f"keyt{i}") for i in range(2)]
        for i in range(2):
            nc.sync.dma_start(wkt[i][:], dram["wkt"][i * 128:(i + 1) * 128, :])
            nc.sync.dma_start(keyt[i][:], dram["keyt"][i * 128:(i + 1) * 128, :])
        cts = []
        for i in range(9):
            r = 128 if i < 8 else 2
            t = const.tile([r, NQH], fp16, tag=f"ct{i}", name=f"ct{i}")
            nc.sync.dma_start(t[:], dram["ct"][i * 128:i * 128 + r, :])
            cts.append(t)
        keyn = [const.tile([128, D], bf16, tag=f"keyn{i}", name=f"keyn{i}") for i in range(NKT)]
        for i in range(NKT):
            nc.sync.dma_start(keyn[i][:], dram["keyn"][i * 128:(i + 1) * 128, :])
        id_b = const.tile([128, 128], bf16, tag="ident", name="ident")
        nc.sync.dma_start(id_b[:], dram["ident"][:])

        # ---- phase A: z = y @ Wq^T (9 row tiles; chains interleaved 4-5 wide)
        zbuf = []
        for i in range(8):
            zbuf.append(zpool.tile([128, D], fp16, tag=f"z{i}", name=f"z{i}"))
        zbuf.append(zpool.tile([2, D], fp16, tag="z8", name="z8"))  # row0: z[1024], row1: bq
        nc.sync.dma_start(zbuf[8][1:2, :], dram["bq"][:])

        with tc.tile_pool(name="psA", bufs=5, space="PSUM") as psA:
            for grp in (range(0, 5), range(5, 9)):
                pss = {}
                for nt in grp:
                    pss[nt] = psA.tile([128, D], f32, tag="psA", name="psA")
                for kd in range(2):
                    for nt in grp:
                        m = 128 if nt < 8 else 1
                        nc.tensor.matmul(
                            pss[nt][:m, :], yt[kd][:, nt * 128:nt * 128 + m],
                            wqt[kd][:], start=(kd == 0), stop=(kd == 1))
                for nt in grp:
                    m = 128 if nt < 8 else 1
                    nc.vector.tensor_copy(zbuf[nt][:m, :], pss[nt][:m, :])

        # ---- phase C: kT = Wk @ key^T  [256, 2048]; 8 chains interleaved ----
        kT = [qkpool.tile([128, NSEQ], fp16, tag=f"kT{i}", name=f"kT{i}") for i in range(2)]
        with tc.tile_pool(name="psC", bufs=8, space="PSUM") as psC:
            pss = {}
            for dt in range(2):
                for c in range(4):
                    pss[(dt, c)] = psC.tile([128, 512], f32, tag="psC", name="psC")
            for kd in range(2):
                for dt in range(2):
                    for c in range(4):
                        sl = slice(c * 512, (c + 1) * 512)
                        nc.tensor.matmul(
                            pss[(dt, c)][:], wkt[kd][:, dt * 128:(dt + 1) * 128],
                            keyt[kd][:, sl], start=(kd == 0), stop=(kd == 1))
            for dt in range(2):
                for c in range(4):
                    sl = slice(c * 512, (c + 1) * 512)
                    nc.vector.tensor_copy(kT[dt][:, sl], pss[(dt, c)][:])

        # ---- phase B: qsT = z^T @ (C/16)  [256, 1152]; 6 chains interleaved --
        qsT = [qkpool.tile([128, NQH], fp16, tag=f"qsT{i}", name=f"qsT{i}") for i in range(2)]
        with tc.tile_pool(name="psB", bufs=6, space="PSUM") as psB:
            pss = {}
            for dt in range(2):
                for c in range(3):
                    pss[(dt, c)] = psB.tile([128, 384], f32, tag="psB", name="psB")
            for kt in range(9):
                kr = 128 if kt < 8 else 2
                for dt in range(2):
                    for c in range(3):
                        sl = slice(c * 384, (c + 1) * 384)
                        nc.tensor.matmul(
                            pss[(dt, c)][:], zbuf[kt][:kr, dt * 128:(dt + 1) * 128],
                            cts[kt][:kr, sl], start=(kt == 0), stop=(kt == 8))
            for dt in range(2):
                for c in range(3):
                    sl = slice(c * 384, (c + 1) * 384)
                    nc.vector.tensor_copy(qsT[dt][:, sl], pss[(dt, c)][:])

        # ---- phase D: attention over 9 query tiles, software-pipelined ----
        with ExitStack() as jctx:
            psS = jctx.enter_context(tc.tile_pool(name="psS", bufs=2, space="PSUM"))
            psT = jctx.enter_context(tc.tile_pool(name="psT", bufs=2, space="PSUM"))
            psO = jctx.enter_context(tc.tile_pool(name="psO", bufs=2, space="PSUM"))
            work = jctx.enter_context(tc.tile_pool(name="work", bufs=2))
            ptp = jctx.enter_context(tc.tile_pool(name="ptp", bufs=6))
            stats = jctx.enter_context(tc.tile_pool(name="stats", bufs=4))

            state = {}  # per-jt carried tiles
            for step in range(NJT + 1):
                if step < NJT:
                    jt = step
                    jsl = slice(jt * 128, (jt + 1) * 128)
                    # scores in two 1024-wide halves (2 psum banks each);
                    # within a half the two 512-chunks interleave the K steps
                    halves = []
                    for h in range(2):
                        sh = psS.tile([128, 1024], f32, tag="psS", name="psS")
                        for dt in range(2):
                            for c in range(2):
                                sl = slice(c * 512, (c + 1) * 512)
                                ksl = slice(h * 1024 + c * 512, h * 1024 + (c + 1) * 512)
                                nc.tensor.matmul(
                                    sh[:, sl], qsT[dt][:, jsl], kT[dt][:, ksl],
                                    start=(dt == 0), stop=(dt == 1))
                        halves.append(sh)
                    mx = [stats.tile([128, 1], f32, tag=f"mx{h}", name=f"mx{h}") for h in range(2)]
                    for h in range(2):
                        nc.vector.reduce_max(out=mx[h][:], in_=halves[h][:],
                                             axis=mybir.AxisListType.X)
                    rmax = stats.tile([128, 1], f32, tag="rmax", name="rmax")
                    nc.vector.tensor_scalar_max(rmax[:], mx[0][:], mx[1][:])
                    negmax = stats.tile([128, 1], f32, tag="negmax", name="negmax")
                    nc.scalar.mul(negmax[:], rmax[:], -1.0)
                    p_t = work.tile([128, NSEQ], bf16, tag="p", name="p")
                    sm = [stats.tile([128, 1], f32, tag=f"sm{h}", name=f"sm{h}") for h in range(2)]
                    for h in range(2):
                        nc.scalar.activation(
                            out=p_t[:, h * 1024:(h + 1) * 1024], in_=halves[h][:],
                            func=mybir.ActivationFunctionType.Exp,
                            bias=negmax[:], scale=1.0, accum_out=sm[h][:])
                    rsum = stats.tile([128, 1], f32, tag="rsum", name="rsum")
                    nc.vector.tensor_scalar_add(rsum[:], sm[0][:], sm[1][:])
                    recip = stats.tile([128, 1], f32, tag="recip", name="recip")
                    nc.vector.reciprocal(recip[:], rsum[:])
                    state[jt] = (p_t, recip)

                if step >= 1:
                    jt = step - 1
                    p_t, recip = state.pop(jt)
                    # two interleaved accumulation chains (even/odd key tiles)
                    po = [psO.tile([128, D], f32, tag="psO", name="psO")
                          for _ in range(2)]
                    for kt in range(NKT):
                        pt_ps = psT.tile([128, 128], bf16, tag="psT", name="psT")
                        nc.tensor.matmul(pt_ps[:], p_t[:, kt * 128:(kt + 1) * 128],
                                         id_b[:], is_transpose=True,
                                         start=True, stop=True)
                        pt_sb = ptp.tile([128, 128], bf16, tag="pt", name="pt")
                        nc.vector.tensor_copy(pt_sb[:], pt_ps[:])
                        nc.tensor.matmul(po[kt % 2][:], pt_sb[:], keyn[kt][:],
                                         start=(kt < 2), stop=(kt >= NKT - 2))
                    osb0 = work.tile([128, D], f32, tag="osb0", name="osb0")
                    nc.vector.tensor_scalar_mul(osb0[:], po[0][:], recip[:])
                    osb = work.tile([128, D], f32, tag="osb", name="osb")
                    nc.vector.scalar_tensor_tensor(
                        out=osb[:], in0=po[1][:], scalar=recip[:], in1=osb0[:],
                        op0=mybir.AluOpType.mult, op1=mybir.AluOpType.add)
                    nc.sync.dma_start(dram["ob"][jt * 128:(jt + 1) * 128, :], osb[:])


def _host_prep(query, key, Wq, bq, Wk, bk):
    """Build per-core input maps (fold+transpose query, transpose key/weights,
    cosine table)."""
    query = np.ascontiguousarray(query, dtype=np.float32)
    key = np.ascontiguousarray(key, dtype=np.float32)

    nn = np.arange(NFOLD - 1, dtype=np.float64)          # 0..1024
    jj = np.arange(NQH, dtype=np.float64)
    ct = np.empty((NFOLD, NQH), dtype=np.float32)
    ct[:-1] = (np.cos(2.0 * np.pi * np.outer(nn, jj) / NSEQ) * SCALE).astype(np.float32)
    ct[-1] = SCALE  # bias row (ones * scale)
    ct = ct.astype(ml_dtypes.float16) if False else ct.astype(np.float16)

    wqt = np.ascontiguousarray(Wq.T).astype(np.float16)
    wkt = np.ascontiguousarray(Wk.T).astype(np.float16)
    bq2 = np.ascontiguousarray(bq.reshape(1, D)).astype(np.float16)

    in_maps = []
    for b in range(B):
        x = query[b]
        y = np.empty((1025, D), dtype=np.float32)
        y[0] = x[0]
        y[1:1024] = x[1:1024] + x[2047:1024:-1]
        y[1024] = x[1024]
        in_maps.append({
            "yt": np.ascontiguousarray(y.T).astype(np.float16),
            "bq": bq2,
            "wqt": wqt,
            "wkt": wkt,
            "keyt": np.ascontiguousarray(key[b].T).astype(np.float16),
            "keyn": np.ascontiguousarray(key[b]).astype(ml_dtypes.bfloat16),
            "ct": ct,
            "ident": np.eye(128, dtype=ml_dtypes.bfloat16),
        })
    return in_maps


def kernel(query, key, Wq, bq, Wk, bk, _trace=False, _trace_kwargs=None):
    if "nc" not in _compiled:
        _compiled["nc"] = _build_module()
    nc = _compiled["nc"]

    in_maps = _host_prep(query, key, Wq, bq, Wk, bk)
    kw = {}
    if _trace:
        kw["trace"] = True
        if _trace_kwargs:
            kw.update(_trace_kwargs)
    res = run_bass_kernel_spmd(nc, in_maps, core_ids=list(range(B)), **kw)
    _compiled["last_results"] = res

    out = np.empty((B, NSEQ, D), dtype=np.float32)
    for b in range(B):
        ob = res.results[b]["ob"]
        out[b, :1025] = ob[:1025]
        out[b, 1025:] = ob[1023:0:-1]
    return out
